# revision 1
# baseline (speedup 1.0000x reference)
"""Trainium2 Bass kernel for nn_AdjEnsemble (gnn_message_passing).

Math: softmax rows of adj sum to 1, so adj_norm @ (sv - c_k) = adj_norm@sv - c_k.
With E = exp(-adj) (no max-subtraction needed: adj ~ N(0,1)) and
R = rowsum(E), the whole module collapses to

    t        = (E @ sv) / R                      # [N, D]
    features = mean_k lrelu(t - c_k)
             = 0.2475 * sum_k relu(t - c_k) + 0.01*(t - mean_k c_k)
    out      = relu(features @ fc_w.T + fc_b)    # [N, OUT]

Sharding: adj rows split across 8 cores ([1024, 8192] each); everything
else replicated. No collectives: each core's output rows stay local.

Per-core dataflow: DMA adj tiles (natural layout, f32) -> PE transpose
128x128 blocks into PSUM -> ACT exp(-x) PSUM->SBUF (bf16) -> PE matmul
accumulate S^T[65, m] = svaug^T @ E^T over n-chunks (svaug has a ones
column so row 64 of S^T is the softmax denominator R) -> small epilogue
on DVE -> fc matmul (bias folded in as an extra contraction row) -> relu
-> DMA out.
"""

import numpy as np
import ml_dtypes

_BF16 = ml_dtypes.bfloat16

N, D, K, OUT = 8192, 64, 4, 256
N_CORES = 8
M_SH = N // N_CORES          # 1024 adj rows per core
DA = D + 1                   # 65: sv columns + ones column
MSUP = 512                   # m rows accumulated per PSUM accumulator
NBLK = 2048                  # n columns per DMA'd adj tile
NCH = 128                    # n contraction chunk (PE partition dim)
LRELU_SLOPE = 0.01
RELU_COEF = (1.0 - LRELU_SLOPE) / 4.0       # 0.2475
G = LRELU_SLOPE / RELU_COEF                 # linear-term coefficient inside feat'

_GRAPH_CACHE = {}


def _build_graph():
    if "nc" in _GRAPH_CACHE:
        return _GRAPH_CACHE["nc"]

    import concourse.tile as tile
    from concourse import bacc, mybir

    f32 = mybir.dt.float32
    bf16 = mybir.dt.bfloat16
    Act = mybir.ActivationFunctionType
    Alu = mybir.AluOpType

    nc = bacc.Bacc("TRN2", target_bir_lowering=False, debug=False,
                   num_devices=N_CORES)

    adj_ext = nc.declare_dram_parameter("adj", [M_SH, N], f32, isOutput=False)
    svp_ext = nc.declare_dram_parameter("svp", [128, (N // NCH) * DA], bf16,
                                        isOutput=False)
    id_ext = nc.declare_dram_parameter("ident", [128, 128], f32, isOutput=False)
    wt_ext = nc.declare_dram_parameter("wt", [DA, OUT], bf16, isOutput=False)
    epi_ext = nc.declare_dram_parameter("epi", [D, 8], f32, isOutput=False)
    out_ext = nc.declare_dram_parameter("out", [M_SH, OUT], f32, isOutput=True)

    n_msup = M_SH // MSUP            # 2
    n_nblk = N // NBLK               # 4
    n_nch = NBLK // NCH              # 16
    n_sub = MSUP // 128              # 4

    with tile.TileContext(nc) as tc:
        with (
            tc.tile_pool(name="const", bufs=1) as const,
            tc.tile_pool(name="a", bufs=2 * n_sub) as apool,
            tc.tile_pool(name="et", bufs=3) as etpool,
            tc.tile_pool(name="epi", bufs=2) as epool,
            tc.tile_pool(name="osb", bufs=2) as outp,
            tc.tile_pool(name="stage", bufs=3, space="PSUM") as stpool,
            tc.tile_pool(name="acc", bufs=2, space="PSUM") as accpool,
            tc.tile_pool(name="bps", bufs=1, space="PSUM") as bpool,
            tc.tile_pool(name="ops", bufs=2, space="PSUM") as opool,
        ):
            svt = const.tile([128, (N // NCH) * DA], bf16)
            nc.sync.dma_start(svt[:], svp_ext[:])
            idt = const.tile([128, 128], f32)
            nc.sync.dma_start(idt[:], id_ext[:])
            wtt = const.tile([DA, OUT], bf16)
            nc.sync.dma_start(wtt[:], wt_ext[:])
            epit = const.tile([D, 8], f32)
            nc.sync.dma_start(epit[:], epi_ext[:])
            onest = const.tile([1, D], f32)
            nc.vector.memset(onest[:], 1.0)

            for ms in range(n_msup):
                m0 = ms * MSUP
                acc = accpool.tile([DA, MSUP], f32)
                for nb in range(n_nblk):
                    ats = []
                    for s in range(n_sub):
                        at = apool.tile([128, NBLK], f32, tag="a")
                        nc.sync.dma_start(
                            at[:],
                            adj_ext[m0 + s * 128:m0 + (s + 1) * 128,
                                    nb * NBLK:(nb + 1) * NBLK])
                        ats.append(at)
                    for nch in range(n_nch):
                        chunk = nb * n_nch + nch
                        stage = stpool.tile([128, MSUP], f32)
                        for s in range(n_sub):
                            nc.tensor.transpose(
                                stage[:, s * 128:(s + 1) * 128],
                                ats[s][:, nch * NCH:(nch + 1) * NCH],
                                idt[:])
                        et = etpool.tile([128, MSUP], bf16)
                        nc.scalar.activation(et[:], stage[:], Act.Exp,
                                             scale=-1.0)
                        nc.tensor.matmul(
                            acc[:],
                            svt[:, chunk * DA:(chunk + 1) * DA],
                            et[:],
                            start=(chunk == 0),
                            stop=(chunk == (N // NCH) - 1))

                # epilogue: acc[0:64] = S^T rows, acc[64] = R (softmax denom)
                rinv = epool.tile([1, MSUP], f32, tag="rinv")
                nc.vector.reciprocal(rinv[:], acc[D:DA, :])
                rb_ps = bpool.tile([D, MSUP], f32)
                nc.tensor.matmul(rb_ps[:], onest[:], rinv[:],
                                 start=True, stop=True)
                rb = epool.tile([D, MSUP], f32, tag="rb")
                nc.vector.tensor_copy(rb[:], rb_ps[:])
                t = epool.tile([D, MSUP], f32, tag="t")
                nc.vector.tensor_tensor(t[:], acc[0:D, :], rb[:], Alu.mult)

                rk = []
                for k in range(K):
                    r = epool.tile([D, MSUP], f32, tag=f"r{k}")
                    nc.vector.tensor_scalar(r[:], t[:], epit[:, k:k + 1], 0.0,
                                            Alu.add, Alu.max)
                    rk.append(r)
                q = epool.tile([D, MSUP], f32, tag="q")
                nc.vector.tensor_scalar(q[:], t[:], G, epit[:, 4:5],
                                        Alu.mult, Alu.add)
                s01 = epool.tile([D, MSUP], f32, tag="s01")
                nc.vector.tensor_tensor(s01[:], rk[0][:], rk[1][:], Alu.add)
                s23 = epool.tile([D, MSUP], f32, tag="s23")
                nc.vector.tensor_tensor(s23[:], rk[2][:], rk[3][:], Alu.add)
                s03 = epool.tile([D, MSUP], f32, tag="s03")
                nc.vector.tensor_tensor(s03[:], s01[:], s23[:], Alu.add)
                feat = epool.tile([DA, MSUP], bf16, tag="feat")
                nc.vector.memset(feat[D:DA, :], 1.0)
                nc.vector.tensor_tensor(feat[0:D, :], s03[:], q[:], Alu.add)

                for mc in range(n_sub):
                    ops = opool.tile([128, OUT], f32)
                    nc.tensor.matmul(ops[:],
                                     feat[:, mc * 128:(mc + 1) * 128],
                                     wtt[:], start=True, stop=True)
                    osb = outp.tile([128, OUT], f32)
                    nc.vector.tensor_scalar_max(osb[:], ops[:], 0.0)
                    nc.sync.dma_start(
                        out_ext[m0 + mc * 128:m0 + (mc + 1) * 128, :],
                        osb[:])

    nc.compile()
    _GRAPH_CACHE["nc"] = nc
    return nc


def _prep_in_maps(semantic_vec, adj, field_centers, fc_w, fc_b):
    svaug = np.concatenate(
        [semantic_vec.astype(np.float32),
         np.ones((N, 1), np.float32)], axis=1)                     # [N, 65]
    svp = np.ascontiguousarray(
        svaug.reshape(N // NCH, NCH, DA).transpose(1, 0, 2)
        .reshape(NCH, (N // NCH) * DA)).astype(_BF16)              # [128, 64*65]
    ident = np.eye(128, dtype=np.float32)
    wt = np.concatenate(
        [RELU_COEF * fc_w.T.astype(np.float32),
         fc_b.astype(np.float32)[None, :]], axis=0).astype(_BF16)  # [65, OUT]
    epi = np.zeros((D, 8), np.float32)
    epi[:, 0:K] = -field_centers.T
    epi[:, 4] = -G * field_centers.mean(axis=0)
    adj = np.ascontiguousarray(adj.astype(np.float32))

    in_maps = []
    for c in range(N_CORES):
        in_maps.append({
            "adj": adj[c * M_SH:(c + 1) * M_SH],
            "svp": svp,
            "ident": ident,
            "wt": wt,
            "epi": epi,
        })
    return in_maps


def run(semantic_vec, adj, field_centers, fc_w, fc_b, trace=False):
    from concourse.bass_utils import run_bass_kernel_spmd

    nc = _build_graph()
    in_maps = _prep_in_maps(semantic_vec, adj, field_centers, fc_w, fc_b)
    res = run_bass_kernel_spmd(nc, in_maps, core_ids=list(range(N_CORES)),
                               trace=trace)
    out = np.concatenate([res.results[i]["out"] for i in range(N_CORES)],
                         axis=0)
    return out, res


def kernel(semantic_vec, adj, field_centers, fc_w, fc_b):
    out, _ = run(semantic_vec, adj, field_centers, fc_w, fc_b, trace=False)
    return out


# revision 7
# speedup vs baseline: 1.0057x; 1.0057x over previous
"""Trainium2 Bass kernel for nn_AdjEnsemble (gnn_message_passing).

Math: softmax rows of adj sum to 1, so adj_norm @ (sv - c_k) = adj_norm@sv - c_k.
With E = exp(-adj) (no max-subtraction needed: adj ~ N(0,1)) and
R = rowsum(E), the whole module collapses to

    t        = (E @ sv) / R                      # [N, D]
    features = mean_k lrelu(t - c_k)
             = 0.2475 * sum_k relu(t - c_k) + 0.01*(t - mean_k c_k)
    out      = relu(features @ fc_w.T + fc_b)    # [N, OUT]

Sharding: adj rows split across 8 cores ([1024, 8192] each); everything
else replicated. No collectives: each core's output rows stay local.

Per-core dataflow: DMA adj tiles (natural layout, f32) -> PE transpose
128x128 blocks into PSUM -> ACT exp(-x) PSUM->SBUF (bf16) -> PE matmul
accumulate S^T[65, m] = svaug^T @ E^T over n-chunks (svaug has a ones
column so row 64 of S^T is the softmax denominator R) -> small epilogue
on DVE -> fc matmul (bias folded in as an extra contraction row) -> relu
-> DMA out.
"""

import numpy as np
import ml_dtypes

_BF16 = ml_dtypes.bfloat16

N, D, K, OUT = 8192, 64, 4, 256
N_CORES = 8
M_SH = N // N_CORES          # 1024 adj rows per core
DA = D + 1                   # 65: sv columns + ones column
MSUP = 512                   # m rows accumulated per PSUM accumulator
NBLK = 2048                  # n columns per DMA'd adj tile
NCH = 128                    # n contraction chunk (PE partition dim)
LRELU_SLOPE = 0.01
RELU_COEF = (1.0 - LRELU_SLOPE) / 4.0       # 0.2475
G = LRELU_SLOPE / RELU_COEF                 # linear-term coefficient inside feat'

_GRAPH_CACHE = {}


def _build_graph():
    if "nc" in _GRAPH_CACHE:
        return _GRAPH_CACHE["nc"]

    import concourse.tile as tile
    from concourse import bacc, mybir

    f32 = mybir.dt.float32
    bf16 = mybir.dt.bfloat16
    Act = mybir.ActivationFunctionType
    Alu = mybir.AluOpType

    nc = bacc.Bacc("TRN2", target_bir_lowering=False, debug=False,
                   num_devices=N_CORES)

    adj_ext = nc.declare_dram_parameter("adj", [M_SH, N], f32, isOutput=False)
    svp_ext = nc.declare_dram_parameter("svp", [128, (N // NCH) * DA], bf16,
                                        isOutput=False)
    id_ext = nc.declare_dram_parameter("ident", [128, 128], bf16, isOutput=False)
    wt_ext = nc.declare_dram_parameter("wt", [DA, OUT], bf16, isOutput=False)
    epi_ext = nc.declare_dram_parameter("epi", [D, 8], f32, isOutput=False)
    out_ext = nc.declare_dram_parameter("out", [M_SH, OUT], f32, isOutput=True)

    n_msup = M_SH // MSUP            # 2
    n_nblk = N // NBLK               # 4
    n_nch = NBLK // NCH              # 16
    n_sub = MSUP // 128              # 4

    with tile.TileContext(nc) as tc:
        with (
            tc.tile_pool(name="const", bufs=1) as const,
            tc.tile_pool(name="a", bufs=2 * n_sub) as apool,
            tc.tile_pool(name="et", bufs=3) as etpool,
            tc.tile_pool(name="epi", bufs=2) as epool,
            tc.tile_pool(name="osb", bufs=2) as outp,
            tc.tile_pool(name="stage", bufs=2, space="PSUM") as stpool,
            tc.tile_pool(name="acc", bufs=2, space="PSUM") as accpool,
            tc.tile_pool(name="bps", bufs=1, space="PSUM") as bpool,
            tc.tile_pool(name="ops", bufs=1, space="PSUM") as opool,
        ):
            svt = const.tile([128, (N // NCH) * DA], bf16)
            nc.sync.dma_start(svt[:], svp_ext[:])
            idt = const.tile([128, 128], bf16)
            nc.sync.dma_start(idt[:], id_ext[:])
            wtt = const.tile([DA, OUT], bf16)
            nc.sync.dma_start(wtt[:], wt_ext[:])
            epit = const.tile([D, 8], f32)
            nc.sync.dma_start(epit[:], epi_ext[:])
            onest = const.tile([1, D], f32)
            nc.vector.memset(onest[:], 1.0)

            for ms in range(n_msup):
                m0 = ms * MSUP
                acc = accpool.tile([DA, MSUP], f32)
                for nb in range(n_nblk):
                    ats = []
                    for s in range(n_sub):
                        at = apool.tile([128, NBLK], bf16, tag="a")
                        # gpsimd (SWDGE) DMA casts f32 DRAM -> bf16 SBUF
                        nc.gpsimd.dma_start(
                            at[:],
                            adj_ext[m0 + s * 128:m0 + (s + 1) * 128,
                                    nb * NBLK:(nb + 1) * NBLK])
                        ats.append(at)
                    for nch2 in range(n_nch // 2):
                        stage = stpool.tile([128, 2 * MSUP], bf16)
                        for j in range(2):
                            nch = nch2 * 2 + j
                            for s in range(n_sub):
                                nc.tensor.transpose(
                                    stage[:, j * MSUP + s * 128:
                                          j * MSUP + (s + 1) * 128],
                                    ats[s][:, nch * NCH:(nch + 1) * NCH],
                                    idt[:])
                        et = etpool.tile([128, 2 * MSUP], bf16)
                        nc.scalar.activation(et[:], stage[:], Act.Exp,
                                             scale=-1.0)
                        for j in range(2):
                            chunk = nb * n_nch + nch2 * 2 + j
                            nc.tensor.matmul(
                                acc[:],
                                svt[:, chunk * DA:(chunk + 1) * DA],
                                et[:, j * MSUP:(j + 1) * MSUP],
                                start=(chunk == 0),
                                stop=(chunk == (N // NCH) - 1))

                # epilogue: acc[0:64] = S^T rows, acc[64] = R (softmax denom)
                rinv = epool.tile([1, MSUP], f32, tag="rinv")
                nc.vector.reciprocal(rinv[:], acc[D:DA, :])
                rb_ps = bpool.tile([D, MSUP], f32)
                nc.tensor.matmul(rb_ps[:], onest[:], rinv[:],
                                 start=True, stop=True)
                rb = epool.tile([D, MSUP], f32, tag="rb")
                nc.vector.tensor_copy(rb[:], rb_ps[:])
                t = epool.tile([D, MSUP], f32, tag="t")
                nc.vector.tensor_tensor(t[:], acc[0:D, :], rb[:], Alu.mult)

                rk = []
                for k in range(K):
                    r = epool.tile([D, MSUP], f32, tag=f"r{k}")
                    nc.vector.tensor_scalar(r[:], t[:], epit[:, k:k + 1], 0.0,
                                            Alu.add, Alu.max)
                    rk.append(r)
                q = epool.tile([D, MSUP], f32, tag="q")
                nc.vector.tensor_scalar(q[:], t[:], G, epit[:, 4:5],
                                        Alu.mult, Alu.add)
                s01 = epool.tile([D, MSUP], f32, tag="s01")
                nc.vector.tensor_tensor(s01[:], rk[0][:], rk[1][:], Alu.add)
                s23 = epool.tile([D, MSUP], f32, tag="s23")
                nc.vector.tensor_tensor(s23[:], rk[2][:], rk[3][:], Alu.add)
                s03 = epool.tile([D, MSUP], f32, tag="s03")
                nc.vector.tensor_tensor(s03[:], s01[:], s23[:], Alu.add)
                feat = epool.tile([DA, MSUP], bf16, tag="feat")
                nc.vector.memset(feat[D:DA, :], 1.0)
                nc.vector.tensor_tensor(feat[0:D, :], s03[:], q[:], Alu.add)

                for mc in range(n_sub):
                    ops = opool.tile([128, OUT], f32)
                    nc.tensor.matmul(ops[:],
                                     feat[:, mc * 128:(mc + 1) * 128],
                                     wtt[:], start=True, stop=True)
                    osb = outp.tile([128, OUT], f32)
                    nc.vector.tensor_scalar_max(osb[:], ops[:], 0.0)
                    nc.sync.dma_start(
                        out_ext[m0 + mc * 128:m0 + (mc + 1) * 128, :],
                        osb[:])

    nc.compile()
    _GRAPH_CACHE["nc"] = nc
    return nc


def _prep_in_maps(semantic_vec, adj, field_centers, fc_w, fc_b):
    svaug = np.concatenate(
        [semantic_vec.astype(np.float32),
         np.ones((N, 1), np.float32)], axis=1)                     # [N, 65]
    svp = np.ascontiguousarray(
        svaug.reshape(N // NCH, NCH, DA).transpose(1, 0, 2)
        .reshape(NCH, (N // NCH) * DA)).astype(_BF16)              # [128, 64*65]
    ident = np.eye(128, dtype=np.float32).astype(_BF16)
    wt = np.concatenate(
        [RELU_COEF * fc_w.T.astype(np.float32),
         fc_b.astype(np.float32)[None, :]], axis=0).astype(_BF16)  # [65, OUT]
    epi = np.zeros((D, 8), np.float32)
    epi[:, 0:K] = -field_centers.T
    epi[:, 4] = -G * field_centers.mean(axis=0)
    adj = np.ascontiguousarray(adj.astype(np.float32))

    in_maps = []
    for c in range(N_CORES):
        in_maps.append({
            "adj": adj[c * M_SH:(c + 1) * M_SH],
            "svp": svp,
            "ident": ident,
            "wt": wt,
            "epi": epi,
        })
    return in_maps


def run(semantic_vec, adj, field_centers, fc_w, fc_b, trace=False):
    from concourse.bass_utils import run_bass_kernel_spmd

    nc = _build_graph()
    in_maps = _prep_in_maps(semantic_vec, adj, field_centers, fc_w, fc_b)
    res = run_bass_kernel_spmd(nc, in_maps, core_ids=list(range(N_CORES)),
                               trace=trace)
    out = np.concatenate([res.results[i]["out"] for i in range(N_CORES)],
                         axis=0)
    return out, res


def kernel(semantic_vec, adj, field_centers, fc_w, fc_b):
    out, _ = run(semantic_vec, adj, field_centers, fc_w, fc_b, trace=False)
    return out


# revision 9
# speedup vs baseline: 1.0065x; 1.0008x over previous
"""Trainium2 Bass kernel for nn_AdjEnsemble (gnn_message_passing).

Math: softmax rows of adj sum to 1, so adj_norm @ (sv - c_k) = adj_norm@sv - c_k.
With E = exp(-adj) (no max-subtraction needed: adj ~ N(0,1)) and
R = rowsum(E), the whole module collapses to

    t        = (E @ sv) / R                      # [N, D]
    features = mean_k lrelu(t - c_k)
             = 0.2475 * sum_k relu(t - c_k) + 0.01*(t - mean_k c_k)
    out      = relu(features @ fc_w.T + fc_b)    # [N, OUT]

Sharding: adj rows split across 8 cores ([1024, 8192] each); everything
else replicated. No collectives: each core's output rows stay local.

Per-core dataflow: DMA adj tiles (natural layout, f32) -> PE transpose
128x128 blocks into PSUM -> ACT exp(-x) PSUM->SBUF (bf16) -> PE matmul
accumulate S^T[65, m] = svaug^T @ E^T over n-chunks (svaug has a ones
column so row 64 of S^T is the softmax denominator R) -> small epilogue
on DVE -> fc matmul (bias folded in as an extra contraction row) -> relu
-> DMA out.
"""

import numpy as np
import ml_dtypes

_BF16 = ml_dtypes.bfloat16

N, D, K, OUT = 8192, 64, 4, 256
N_CORES = 8
M_SH = N // N_CORES          # 1024 adj rows per core
DA = D + 1                   # 65: sv columns + ones column
MSUP = 512                   # m rows accumulated per PSUM accumulator
NBLK = 2048                  # n columns per DMA'd adj tile
NCH = 128                    # n contraction chunk (PE partition dim)
LRELU_SLOPE = 0.01
RELU_COEF = (1.0 - LRELU_SLOPE) / 4.0       # 0.2475
G = LRELU_SLOPE / RELU_COEF                 # linear-term coefficient inside feat'

_GRAPH_CACHE = {}


def _build_graph():
    if "nc" in _GRAPH_CACHE:
        return _GRAPH_CACHE["nc"]

    import concourse.tile as tile
    from concourse import bacc, mybir

    f32 = mybir.dt.float32
    f32r = mybir.dt.float32r
    bf16 = mybir.dt.bfloat16
    Act = mybir.ActivationFunctionType
    Alu = mybir.AluOpType

    nc = bacc.Bacc("TRN2", target_bir_lowering=False, debug=False,
                   num_devices=N_CORES)

    adj_ext = nc.declare_dram_parameter("adj", [M_SH, N], f32r, isOutput=False)
    svp_ext = nc.declare_dram_parameter("svp", [128, (N // NCH) * DA], bf16,
                                        isOutput=False)
    id_ext = nc.declare_dram_parameter("ident", [128, 128], f32r, isOutput=False)
    wt_ext = nc.declare_dram_parameter("wt", [DA, OUT], bf16, isOutput=False)
    epi_ext = nc.declare_dram_parameter("epi", [D, 8], f32, isOutput=False)
    out_ext = nc.declare_dram_parameter("out", [M_SH, OUT], f32, isOutput=True)

    n_msup = M_SH // MSUP            # 2
    n_nblk = N // NBLK               # 4
    n_nch = NBLK // NCH              # 16
    n_sub = MSUP // 128              # 4

    with tile.TileContext(nc) as tc:
        with (
            tc.tile_pool(name="const", bufs=1) as const,
            tc.tile_pool(name="a", bufs=2 * n_sub) as apool,
            tc.tile_pool(name="et", bufs=3) as etpool,
            tc.tile_pool(name="epi", bufs=2) as epool,
            tc.tile_pool(name="osb", bufs=2) as outp,
            tc.tile_pool(name="stage", bufs=2, space="PSUM") as stpool,
            tc.tile_pool(name="acc", bufs=2, space="PSUM") as accpool,
            tc.tile_pool(name="bps", bufs=1, space="PSUM") as bpool,
            tc.tile_pool(name="ops", bufs=1, space="PSUM") as opool,
        ):
            svt = const.tile([128, (N // NCH) * DA], bf16)
            nc.sync.dma_start(svt[:], svp_ext[:])
            idt = const.tile([128, 128], f32r)
            nc.sync.dma_start(idt[:], id_ext[:])
            wtt = const.tile([DA, OUT], bf16)
            nc.sync.dma_start(wtt[:], wt_ext[:])
            epit = const.tile([D, 8], f32)
            nc.sync.dma_start(epit[:], epi_ext[:])
            onest = const.tile([1, D], f32)
            nc.vector.memset(onest[:], 1.0)

            for ms in range(n_msup):
                m0 = ms * MSUP
                acc = accpool.tile([DA, MSUP], f32)
                for nb in range(n_nblk):
                    ats = []
                    for s in range(n_sub):
                        at = apool.tile([128, NBLK], f32r, tag="a")
                        nc.gpsimd.dma_start(
                            at[:],
                            adj_ext[m0 + s * 128:m0 + (s + 1) * 128,
                                    nb * NBLK:(nb + 1) * NBLK])
                        ats.append(at)
                    for nch2 in range(n_nch // 2):
                        stage = stpool.tile([128, 2 * MSUP], f32r)
                        for j in range(2):
                            nch = nch2 * 2 + j
                            for s in range(n_sub):
                                nc.tensor.transpose(
                                    stage[:, j * MSUP + s * 128:
                                          j * MSUP + (s + 1) * 128],
                                    ats[s][:, nch * NCH:(nch + 1) * NCH],
                                    idt[:])
                        et = etpool.tile([128, 2 * MSUP], bf16)
                        nc.scalar.activation(et[:], stage[:], Act.Exp,
                                             scale=-1.0)
                        for j in range(2):
                            chunk = nb * n_nch + nch2 * 2 + j
                            nc.tensor.matmul(
                                acc[:],
                                svt[:, chunk * DA:(chunk + 1) * DA],
                                et[:, j * MSUP:(j + 1) * MSUP],
                                start=(chunk == 0),
                                stop=(chunk == (N // NCH) - 1))

                # epilogue: acc[0:64] = S^T rows, acc[64] = R (softmax denom)
                rinv = epool.tile([1, MSUP], f32, tag="rinv")
                nc.vector.reciprocal(rinv[:], acc[D:DA, :])
                rb_ps = bpool.tile([D, MSUP], f32)
                nc.tensor.matmul(rb_ps[:], onest[:], rinv[:],
                                 start=True, stop=True)
                rb = epool.tile([D, MSUP], f32, tag="rb")
                nc.vector.tensor_copy(rb[:], rb_ps[:])
                t = epool.tile([D, MSUP], f32, tag="t")
                nc.vector.tensor_tensor(t[:], acc[0:D, :], rb[:], Alu.mult)

                rk = []
                for k in range(K):
                    r = epool.tile([D, MSUP], f32, tag=f"r{k}")
                    nc.vector.tensor_scalar(r[:], t[:], epit[:, k:k + 1], 0.0,
                                            Alu.add, Alu.max)
                    rk.append(r)
                q = epool.tile([D, MSUP], f32, tag="q")
                nc.vector.tensor_scalar(q[:], t[:], G, epit[:, 4:5],
                                        Alu.mult, Alu.add)
                s01 = epool.tile([D, MSUP], f32, tag="s01")
                nc.vector.tensor_tensor(s01[:], rk[0][:], rk[1][:], Alu.add)
                s23 = epool.tile([D, MSUP], f32, tag="s23")
                nc.vector.tensor_tensor(s23[:], rk[2][:], rk[3][:], Alu.add)
                s03 = epool.tile([D, MSUP], f32, tag="s03")
                nc.vector.tensor_tensor(s03[:], s01[:], s23[:], Alu.add)
                feat = epool.tile([DA, MSUP], bf16, tag="feat")
                nc.vector.memset(feat[D:DA, :], 1.0)
                nc.vector.tensor_tensor(feat[0:D, :], s03[:], q[:], Alu.add)

                for mc in range(n_sub):
                    ops = opool.tile([128, OUT], f32)
                    nc.tensor.matmul(ops[:],
                                     feat[:, mc * 128:(mc + 1) * 128],
                                     wtt[:], start=True, stop=True)
                    osb = outp.tile([128, OUT], f32)
                    nc.vector.tensor_scalar_max(osb[:], ops[:], 0.0)
                    nc.sync.dma_start(
                        out_ext[m0 + mc * 128:m0 + (mc + 1) * 128, :],
                        osb[:])

    nc.compile()
    _GRAPH_CACHE["nc"] = nc
    return nc


def _prep_in_maps(semantic_vec, adj, field_centers, fc_w, fc_b):
    svaug = np.concatenate(
        [semantic_vec.astype(np.float32),
         np.ones((N, 1), np.float32)], axis=1)                     # [N, 65]
    svp = np.ascontiguousarray(
        svaug.reshape(N // NCH, NCH, DA).transpose(1, 0, 2)
        .reshape(NCH, (N // NCH) * DA)).astype(_BF16)              # [128, 64*65]
    ident = np.eye(128, dtype=np.float32)
    wt = np.concatenate(
        [RELU_COEF * fc_w.T.astype(np.float32),
         fc_b.astype(np.float32)[None, :]], axis=0).astype(_BF16)  # [65, OUT]
    epi = np.zeros((D, 8), np.float32)
    epi[:, 0:K] = -field_centers.T
    epi[:, 4] = -G * field_centers.mean(axis=0)
    adj = np.ascontiguousarray(adj.astype(np.float32))

    in_maps = []
    for c in range(N_CORES):
        in_maps.append({
            "adj": adj[c * M_SH:(c + 1) * M_SH],
            "svp": svp,
            "ident": ident,
            "wt": wt,
            "epi": epi,
        })
    return in_maps


def run(semantic_vec, adj, field_centers, fc_w, fc_b, trace=False):
    from concourse.bass_utils import run_bass_kernel_spmd

    nc = _build_graph()
    in_maps = _prep_in_maps(semantic_vec, adj, field_centers, fc_w, fc_b)
    res = run_bass_kernel_spmd(nc, in_maps, core_ids=list(range(N_CORES)),
                               trace=trace)
    out = np.concatenate([res.results[i]["out"] for i in range(N_CORES)],
                         axis=0)
    return out, res


def kernel(semantic_vec, adj, field_centers, fc_w, fc_b):
    out, _ = run(semantic_vec, adj, field_centers, fc_w, fc_b, trace=False)
    return out
